# revision 3
# baseline (speedup 1.0000x reference)
"""Masked L1 loss (per-(b,c) normalized) on 8 Trainium2 NeuronCores.

Layout: batch-dim data parallel. Core i takes batches [2i, 2i+2) of the
[16, 64, 128, 128] inputs -> a [128, 16384] shard (partition = (b, c) pair,
free = h*w). Per [128, 2048] tile:
    DVE  tensor_tensor          sd = pre - gt
    ACT  activation(Abs)        ad = |sd|
    DVE  tensor_tensor_reduce   junk = ad * mask, l1_part[p] = sum(junk)
    ACT  activation(Abs,accum)  junk2 = mask,     ct_part[p] = sum(mask)
Per-(b,c) tile partials land in [128, NT] accumulators, DMA'd to DRAM.
Host: l1 = partials.sum, ct = partials.sum, loss = sum(l1/max(ct,1))/B.
"""

import sys

if "/opt/trn_rl_repo" not in sys.path:
    sys.path.insert(0, "/opt/trn_rl_repo")

import numpy as np

B, C, H, W = 16, 64, 128, 128
N_CORES = 8
BPC = B // N_CORES          # batches per core = 2
P = BPC * C                 # partitions per core = 128 (one (b,c) pair each)
HW = H * W                  # 16384 free elements per partition
T = 2048                    # free-dim tile size
NT = HW // T                # 8 tiles

_CACHE = {}


def _build():
    if "nc" in _CACHE:
        return _CACHE["nc"]

    import concourse.bacc as bacc
    import concourse.mybir as mybir
    from concourse.tile import TileContext

    f32 = mybir.dt.float32
    Alu = mybir.AluOpType
    Act = mybir.ActivationFunctionType

    nc = bacc.Bacc(
        "TRN2",
        target_bir_lowering=False,
        debug=False,
        enable_asserts=False,
        num_devices=N_CORES,
    )

    pre = nc.dram_tensor("pre", [P, HW], f32, kind="ExternalInput").ap()
    gt = nc.dram_tensor("gt", [P, HW], f32, kind="ExternalInput").ap()
    mask = nc.dram_tensor("mask", [P, HW], f32, kind="ExternalInput").ap()
    out = nc.dram_tensor("out", [P, 2 * NT], f32, kind="ExternalOutput").ap()

    with TileContext(nc) as tc:
        with (
            tc.tile_pool(name="io", bufs=3) as io,
            tc.tile_pool(name="work", bufs=3) as work,
            tc.tile_pool(name="acc", bufs=1) as accp,
        ):
            l1p = accp.tile([P, NT], f32, tag="l1p")
            ctp = accp.tile([P, NT], f32, tag="ctp")

            for i in range(NT):
                tp = io.tile([P, T], f32, tag="pre")
                tg = io.tile([P, T], f32, tag="gt")
                tm = io.tile([P, T], f32, tag="mask")
                nc.sync.dma_start(out=tp, in_=pre[:, i * T : (i + 1) * T])
                nc.sync.dma_start(out=tg, in_=gt[:, i * T : (i + 1) * T])
                nc.sync.dma_start(out=tm, in_=mask[:, i * T : (i + 1) * T])

                sd = work.tile([P, T], f32, tag="sd")
                ad = work.tile([P, T], f32, tag="ad")

                nc.vector.tensor_tensor(out=sd, in0=tp, in1=tg, op=Alu.subtract)
                nc.scalar.activation(out=ad, in_=sd, func=Act.Abs)
                # one DVE pass: junk = ad * mask, l1 partial = sum(junk)
                nc.vector.scalar_tensor_tensor(
                    out=sd,
                    in0=ad,
                    scalar=0.0,
                    in1=tm,
                    op0=Alu.bypass,
                    op1=Alu.mult,
                    accum_out=l1p[:, i : i + 1],
                )
                # mask is 0/1 so sum(mask) == nonzero count
                nc.vector.tensor_reduce(
                    out=ctp[:, i : i + 1],
                    in_=tm,
                    axis=mybir.AxisListType.X,
                    op=Alu.add,
                )

            nc.sync.dma_start(out=out[:, 0:NT], in_=l1p)
            nc.sync.dma_start(out=out[:, NT : 2 * NT], in_=ctp)

    nc.compile()
    _CACHE["nc"] = nc
    return nc


def _shard(pre, gt, mask):
    in_maps = []
    for i in range(N_CORES):
        sl = slice(i * BPC, (i + 1) * BPC)
        in_maps.append(
            {
                "pre": np.ascontiguousarray(pre[sl], dtype=np.float32).reshape(P, HW),
                "gt": np.ascontiguousarray(gt[sl], dtype=np.float32).reshape(P, HW),
                "mask": np.ascontiguousarray(mask[sl], dtype=np.float32).reshape(P, HW),
            }
        )
    return in_maps


def _combine(results, batch_size):
    total = np.float32(0.0)
    for r in results:
        o = np.asarray(r["out"], dtype=np.float32)
        l1 = o[:, :NT].sum(axis=1, dtype=np.float32)
        ct = o[:, NT:].sum(axis=1, dtype=np.float32)
        total += (l1 / np.maximum(ct, np.float32(1.0))).sum(dtype=np.float32)
    return np.asarray(total / np.float32(int(batch_size)), dtype=np.float32)


def run(pre, gt, mask, batch_size, trace=False, **bass_kwargs):
    from concourse.bass_utils import run_bass_kernel_spmd

    nc = _build()
    in_maps = _shard(np.asarray(pre), np.asarray(gt), np.asarray(mask))
    res = run_bass_kernel_spmd(
        nc, in_maps, list(range(N_CORES)), trace=trace, **bass_kwargs
    )
    loss = _combine(res.results, batch_size)
    return loss, res


def kernel(pre, gt, mask, batch_size):
    loss, _ = run(pre, gt, mask, batch_size)
    return loss
